# revision 23
# baseline (speedup 1.0000x reference)
"""MoE routing kernel for Trainium2 (8 NeuronCores, SPMD).

Math (faithful to the reference, including its quirks):
  logits = x @ gate_w + gate_b                  # [B,S,E]
  weights = softmax(logits, axis=1)             # softmax over the SEQUENCE axis
  top2 values/indices over experts; only experts 0 and 1 are ever evaluated
  (the reference loops `for ind in range(top_k)` and uses expert `ind`).
  out[t] = c0[t]*eo_0[t] + c1[t]*eo_1[t], where
  eo_e = softmax_D(gelu(x@w1[e]+b1[e]) @ w2[e] + b2[e]) and c_e[t] is the
  top-2 gate weight when expert e is in token t's top-2, else 0.

Sharding: routing + dispatch on host (0.4% of FLOPs). Only tokens whose
top-2 contains expert 0/1 are computed (~25% each). Cores 0-3 handle
expert 0's tokens, cores 4-7 expert 1's, so each core streams only one
expert's weights.

Device: phase A = fp16 GEMM + fused gelu, feature-major. Phase B =
fp8e4 DoubleRow GEMM + fused exp in TOKEN-major orientation: h is the
stationary operand and w2 the moving one, so each DoubleRow LDWEIGHTS
(the serial cost in DR mode) is shared by two 512-wide matmuls
(explicit nc.tensor.ldweights + matmuls with ldweights=False). The
softmax bias folds out on the host: exp(z+b2) = exp(z)*exp(b2), and
the denominator + gate coefficient also apply host-side during the
unshard. Weights are host-packed into the exact SBUF layout.
"""

import sys

import numpy as np

sys.path.insert(0, "/opt/trn_rl_repo")

import concourse.bacc as bacc  # noqa: E402
import concourse.tile as tile  # noqa: E402
from concourse import mybir  # noqa: E402
from concourse.bass_utils import run_bass_kernel_spmd  # noqa: E402

P = 128
D = 1024
F = 4096
KD = D // P  # 8
KF = F // P  # 32
NCORES = 8
CHUNK = 512  # max matmul moving free dim / PSUM bank width (f32)
MG = 4  # F-tiles per w1 DMA group
NWARM = 20  # HAM warm-up matmuls (512-wide)
W2SCALE = 512.0  # fp8 phase-B weight pre-scale (undone in exp's scale)
AF = mybir.ActivationFunctionType

_CACHE = {}


def _gating_coeffs(x, gate_w, gate_b):
    """Host replica of the reference gating. Returns c[T,2] float32 where
    c[:,e] is the gate weight if expert e is in the token's top-2 else 0."""
    B, S, _ = x.shape
    x = np.asarray(x, dtype=np.float32)
    logits = x.reshape(B * S, -1) @ np.asarray(gate_w, dtype=np.float32)
    logits = logits.reshape(B, S, -1) + np.asarray(gate_b, dtype=np.float32)
    m = logits.max(axis=1, keepdims=True)
    e = np.exp(logits - m)
    w = e / e.sum(axis=1, keepdims=True)
    wf = w.reshape(B * S, -1)
    top2 = np.argsort(-wf, axis=-1, kind="stable")[:, :2]
    c = np.zeros((B * S, 2), dtype=np.float32)
    for ex in (0, 1):
        sel = (top2 == ex).any(axis=1)
        c[sel, ex] = wf[sel, ex]
    return c


def _build_nc(n, b_fp8):
    """Bass program for one core: n tokens, one expert.

      h = gelu(w1.T @ x.T + b1)        # [F, n] feature-major, fp16
      pout = exp(scale * h.T @ w2)     # [tokens, D] token-major, no bias
                                       # (host multiplies exp(b2), 1/s, c)

    Layouts (packed on host):
      w1g[p, mg*KD*MG*P + k*MG*P + mi*P + j] = w1[k*P+p, (MG*mg+mi)*P + j]
      w2m[p, kf*D + dcol] = W2SCALE * w2[kf*P+p, dcol]   (fp8e4 when b_fp8)
      xg[p, k*n + t] = x[t, k*P + p]
      pout[p, tt*D + dcol] = exp-output for token tt*P+p
    """
    dt = mybir.dt
    f16 = dt.float16
    f8 = dt.float8e4
    f32 = dt.float32
    bdt = f8 if b_fp8 else f16
    NMG = KF // MG  # 8 w1 groups
    GW1 = KD * MG * P  # cols per w1 group (4096)
    NTT = (n + P - 1) // P  # token tiles for phase B
    tts = [(i * P, min(P, n - i * P)) for i in range(NTT)]

    nc = bacc.Bacc()
    xg = nc.dram_tensor("xg", [P, KD * n], f16, kind="ExternalInput")
    w1d = nc.dram_tensor("w1g", [P, NMG * GW1], f16, kind="ExternalInput")
    w2d = nc.dram_tensor("w2m", [P, KF * D], bdt, kind="ExternalInput")
    b1d = nc.dram_tensor("b1t", [P, KF], f32, kind="ExternalInput")
    pd = nc.dram_tensor("pout", [P, NTT * D], f16, kind="ExternalOutput")

    assert n <= CHUNK, "single-chunk token axis assumed"
    with tile.TileContext(nc) as tc:
        with (
            tc.tile_pool(name="const", bufs=1) as const,
            tc.tile_pool(name="acts", bufs=1) as acts,
            tc.tile_pool(name="wt", bufs=NMG) as wt,
            tc.tile_pool(name="ps", bufs=4, space="PSUM") as ps,
        ):
            warm = const.tile([P, CHUNK], f16)
            nc.gpsimd.memset(warm[:], 0.0)

            # --- input DMAs: few, large, spread over sequencer queues ---
            xs = acts.tile([P, KD * n], f16)
            nc.sync.dma_start(xs[:, : n // 2], xg[:, : n // 2])
            nc.gpsimd.dma_start(xs[:, n // 2 : n], xg[:, n // 2 : n])
            w1t = [wt.tile([P, GW1], f16, tag="w", name=f"w1_{g}") for g in range(NMG)]
            nc.sync.dma_start(w1t[0][:, : GW1 // 2], w1d[:, : GW1 // 2])
            nc.gpsimd.dma_start(w1t[0][:, GW1 // 2 :], w1d[:, GW1 // 2 : GW1])
            nc.gpsimd.dma_start(xs[:, n : 2 * n], xg[:, n : 2 * n])
            nc.sync.dma_start(xs[:, 2 * n : 4 * n], xg[:, 2 * n : 4 * n])
            nc.sync.dma_start(xs[:, 4 * n : 6 * n], xg[:, 4 * n : 6 * n])
            nc.sync.dma_start(xs[:, 6 * n :], xg[:, 6 * n :])
            # w1 g1 jumps the remaining x slices on gpsimd: it is needed
            # right when the m-streaming loop starts
            for g in range(1, NMG):
                eng = nc.gpsimd if g % 2 == 1 else nc.sync
                eng.dma_start(w1t[g][:], w1d[:, g * GW1 : (g + 1) * GW1])
            b1t = const.tile([P, KF], f32)
            nc.scalar.dma_start(b1t[:], b1d[:])
            # w2 tiles reuse w1 group 0/1 buffers (tag "w") => their DMAs
            # self-pace on a WAR dep (wait until those groups' matmuls done)
            w2t = [
                wt.tile([P, KF // 2, D], bdt, tag="w", name=f"w2_{g}")
                for g in range(2)
            ]
            for g in range(2):
                for q in range(2):
                    qs = KF // 4
                    nc.scalar.dma_start(
                        w2t[g][:, q * qs : (q + 1) * qs, :],
                        w2d[:, (g * 2 + q) * qs * D : (g * 2 + q + 1) * qs * D],
                    )

            h = acts.tile([P, KF, n], f16 if not b_fp8 else f8)
            p = acts.tile([P, NTT * D], f16)

            # --- HAM warm-up: prime the PE clock while the first DMAs land
            warm_ps = ps.tile([P, 2 * CHUNK], f32, tag="ps", name="warm_ps")
            for _ in range(NWARM):
                nc.tensor.matmul(
                    warm_ps[:, :CHUNK], warm[:, :P], warm[:], start=True, stop=True
                )
            warm_out = const.tile([1, 1], f32)
            nc.vector.tensor_copy(warm_out[:], warm_ps[0:1, 0:1])

            # --- Phase A: h = gelu(w1.T @ x.T + b1), fp16, feature-major ---
            def act_a(m, pst, half):
                nc.scalar.activation(
                    h[:, m, :],
                    pst[:, half * CHUNK : half * CHUNK + n],
                    AF.Gelu,
                    bias=b1t[:, m : m + 1],
                )

            g0_ps = [ps.tile([P, 2 * CHUNK], f32, tag="ps", name=f"psa0_{i}") for i in range(2)]
            for k in range(KD):
                for mi in range(MG):
                    nc.tensor.matmul(
                        g0_ps[mi // 2][:, (mi % 2) * CHUNK : (mi % 2) * CHUNK + n],
                        w1t[0][:, k * MG * P + mi * P : k * MG * P + (mi + 1) * P],
                        xs[:, k * n : k * n + n],
                        start=(k == 0),
                        stop=(k == KD - 1),
                    )
            for mi in range(MG):
                act_a(mi, g0_ps[mi // 2], mi % 2)

            for mp in range(2, KF // 2):
                pst = ps.tile([P, 2 * CHUNK], f32, tag="ps", name=f"psa_{mp}")
                for half in range(2):
                    m = 2 * mp + half
                    mg, mi = m // MG, m % MG
                    for k in range(KD):
                        nc.tensor.matmul(
                            pst[:, half * CHUNK : half * CHUNK + n],
                            w1t[mg][:, k * MG * P + mi * P : k * MG * P + (mi + 1) * P],
                            xs[:, k * n : k * n + n],
                            start=(k == 0),
                            stop=(k == KD - 1),
                        )
                    act_a(m, pst, half)

            # --- Phase B: pout = exp(scale * h.T @ w2), token-major ---
            # Stationary = h token-tile pair [128, 2, tsz]; moving = w2
            # [128, 2, 512]. One DoubleRow LDWEIGHTS serves both 512-wide
            # d-chunk matmuls (ldweights=False skips the implicit reload).
            kstep = 2 if b_fp8 else 1
            pmode = mybir.MatmulPerfMode.DoubleRow if b_fp8 else None
            escale = 1.0 / W2SCALE if b_fp8 else 1.0
            NCH = D // CHUNK  # 2 moving chunks of 512
            for it, (t0, tsz) in enumerate(tts):
                pst = ps.tile([P, 2 * CHUNK], f32, tag="ps", name=f"psb_{it}")
                for kf in range(0, KF, kstep):
                    g, lkf = kf // (KF // 2), kf % (KF // 2)
                    if b_fp8:
                        wslab = nc.tensor.ldweights(
                            h[:, kf : kf + 2, t0 : t0 + tsz], perf_mode=pmode
                        )
                        for c in range(NCH):
                            mm = nc.tensor.matmul(
                                pst[:tsz, c * CHUNK : (c + 1) * CHUNK],
                                h[:, kf : kf + 2, t0 : t0 + tsz],
                                w2t[g][:, lkf : lkf + 2, c * CHUNK : (c + 1) * CHUNK],
                                start=(kf == 0),
                                stop=(kf + kstep >= KF),
                                perf_mode=pmode,
                            )
                            mm.ins.ldweights = False
                    else:
                        for c in range(NCH):
                            nc.tensor.matmul(
                                pst[:tsz, c * CHUNK : (c + 1) * CHUNK],
                                h[:, kf, t0 : t0 + tsz],
                                w2t[g][:, lkf, c * CHUNK : (c + 1) * CHUNK],
                                start=(kf == 0),
                                stop=(kf + kstep >= KF),
                            )
                for c in range(NCH):
                    nc.scalar.activation(
                        p[:tsz, it * D + c * CHUNK : it * D + (c + 1) * CHUNK],
                        pst[:tsz, c * CHUNK : (c + 1) * CHUNK],
                        AF.Exp,
                        scale=escale,
                    )
                    eng = nc.sync if (2 * it + c) % 2 == 0 else nc.gpsimd
                    eng.dma_start(
                        pd[:tsz, it * D + c * CHUNK : it * D + (c + 1) * CHUNK],
                        p[:tsz, it * D + c * CHUNK : it * D + (c + 1) * CHUNK],
                    )

    nc.finalize()
    return nc


def _get_nc(n, b_fp8):
    key = (n, b_fp8)
    if key not in _CACHE:
        _CACHE[key] = _build_nc(n, b_fp8)
    return _CACHE[key]


def _pack_w1(w1e):
    # [D, F] -> [P, NMG*GW1] with w1g[p, mg*GW1 + k*MG*P + mi*P + j]
    a = w1e.reshape(KD, P, KF // MG, MG, P)  # [k, p, mg, mi, j]
    return np.ascontiguousarray(
        a.transpose(1, 2, 0, 3, 4).reshape(P, KD * KF * P).astype(np.float16)
    )


def _pack_w2(w2e, b_fp8):
    # [F, D] -> [P, KF*D] with w2m[p, kf*D + dcol] = scale * w2[kf*P+p, dcol]
    a = w2e.reshape(KF, P, D).transpose(1, 0, 2).reshape(P, KF * D)
    if b_fp8:
        import ml_dtypes

        q = np.clip(a * W2SCALE, -240, 240).astype(ml_dtypes.float8_e4m3)
        return np.ascontiguousarray(q)
    return np.ascontiguousarray(a.astype(np.float16))


def kernel(x, gate_w, gate_b, w1, b1, w2, b2, top_k, use_bf16=None,
           b_fp8=True, _trace=False, _tmpdir=None):
    x = np.asarray(x)
    B, S, _ = x.shape
    T = B * S
    assert int(top_k) == 2
    c = _gating_coeffs(x, gate_w, gate_b)

    x_f = np.ascontiguousarray(x.reshape(T, D).astype(np.float32))
    idx = [np.nonzero(c[:, ex])[0] for ex in (0, 1)]  # tokens per expert
    per_core = max((len(idx[0]) + 3) // 4, (len(idx[1]) + 3) // 4, 1)
    n = max(((per_core + 15) // 16) * 16, 64)  # 16-align (fp8 DR AP stride)

    w1 = np.asarray(w1, dtype=np.float32)
    w2 = np.asarray(w2, dtype=np.float32)
    b1 = np.asarray(b1, dtype=np.float32)
    b2 = np.asarray(b2, dtype=np.float32)
    wconv = {ex: (_pack_w1(w1[ex]), _pack_w2(w2[ex], b_fp8)) for ex in (0, 1)}
    b1conv = {ex: np.ascontiguousarray(b1[ex].reshape(KF, P).T) for ex in (0, 1)}
    eb2 = {ex: np.exp(b2[ex]).astype(np.float32) for ex in (0, 1)}

    in_maps = []
    core_tok = []  # per-core real token ids
    for core in range(NCORES):
        ex = core // 4
        part = core % 4
        ids = idx[ex][part * per_core : (part + 1) * per_core]
        core_tok.append(ids)
        xgc = np.zeros((D, n), dtype=np.float32)
        if len(ids):
            xgc[:, : len(ids)] = x_f[ids].T
        xgc = (
            xgc.reshape(KD, P, n).transpose(1, 0, 2).reshape(P, KD * n)
        ).astype(np.float16)
        in_maps.append(
            {
                "xg": np.ascontiguousarray(xgc),
                "w1g": wconv[ex][0],
                "w2m": wconv[ex][1],
                "b1t": b1conv[ex],
            }
        )

    nc = _get_nc(n, b_fp8)
    kw = {}
    if _trace:
        kw = {"trace": True, "tmpdir": _tmpdir}
    res = run_bass_kernel_spmd(nc, in_maps, core_ids=list(range(NCORES)), **kw)
    kernel.last_results = res

    NTT = (n + P - 1) // P
    out = np.zeros((T, D), dtype=np.float32)
    for core in range(NCORES):
        ids = core_tok[core]
        if len(ids) == 0:
            continue
        ex = core // 4
        pr = res.results[core]["pout"].reshape(P, NTT, D).astype(np.float32)
        p_t = pr.transpose(1, 0, 2).reshape(NTT * P, D)[: len(ids)]
        p_t *= eb2[ex][None, :]
        s = p_t.sum(axis=1)
        g = c[ids, ex] / s
        out[ids] += g[:, None] * p_t
    return out.reshape(B, S, D)


kernel.last_results = None
